# revision 1
# baseline (speedup 1.0000x reference)
"""GAT layer (dense formulation) on 8 Trainium2 NeuronCores.

Computation (N=4096 nodes, IN_F=512, OUT_F=64, HEADS=4):
    h = (x @ W).reshape(N, H, F)
    s = h . a_src ; t = h . a_dst            (per node, per head)
    e[i,j,k] = leaky_relu(s[i,k] + t[j,k])   masked by adj[i,j]
    attn = softmax_j(e) ; out = attn @ h

Sharding: output rows i (nodes) are sharded 512/core across 8 cores.
Each core computes the full h = x @ W redundantly (cheap), then handles
its own 512 i-rows: logits laid out [j=partitions, i=free] so that the
softmax contraction over j runs on the PE as  [h_k | 1].T @ exp_tile,
with the ones-column producing the softmax denominator for free.

Per (j-tile, head) unit the whole logit computation is ONE fused custom
DVE op:   out = max(v, alpha*v) + logmask,  v = s_bcast + t[j]
(t per-partition scalar, logmask = (adjT-1)*9e15 in bf16). Then one ACT
exp pass per j-tile ([128, 2048], 4 heads wide) and 4 accumulating PE
matmuls in float32r.
"""

import os

import numpy as np
import ml_dtypes

import concourse.bass as bass
import concourse.mybir as mybir
import concourse.tile as tile
from concourse import bacc, bass_utils
from concourse._compat import get_trn_type

# ---------------------------------------------------------------- constants
N = 4096
IN_F = 512
OUT_F = 64
HEADS = 4
ALPHA = 0.2
NEG_BIG = -9e15
NCORES = 8
SHARD = N // NCORES            # 512 output rows per core
NT = N // 128                  # 32 j-tiles (and n-tiles)
KC = IN_F // 128               # 4 contraction chunks
WCOLS = HEADS * OUT_F + 2 * HEADS   # 264 = [W | Ws | Wt]
HB = OUT_F + 1                 # 65 = per-head [h_k | ones] weight block

F32 = mybir.dt.float32
F32R = mybir.dt.float32r
BF16 = mybir.dt.bfloat16

# PE matmul dtype: float32r (tf32, 1 cyc/row) vs float32 (exact, 4 cyc/row).
# The walrus verifier requires every producer feeding an f32r matmul to be
# declared f32r itself, so the dtype threads through DMA/copy/activation outs.
DT_PE = mybir.dt.float32 if os.environ.get("GAT_PE_F32", "0") == "1" else mybir.dt.float32r
USE_CUSTOM_DVE = os.environ.get("GAT_CUSTOM_DVE", "1") == "1"

# ------------------------------------------------------- custom DVE op
_GAT_OP = None


def _register_custom_op():
    """Register the fused GAT logit op in concourse's custom-DVE registry.

    out[p, n] = max(v, v*s1) + in1[p, n]      with v = in0[p, n] + s0[p]

    in0 = s broadcast tile, s0 = t column (per-partition), s1 = alpha,
    in1 = additive log-mask (bf16, 0 or -9e15).
    Runtime registration only (process-local): appended to dve_ops.OPS so
    bass_utils.dve_table_for_ops can find it at NEFF compile time.
    """
    global _GAT_OP
    if _GAT_OP is not None:
        return _GAT_OP
    from concourse import dve_ops as dop
    from concourse.dve_spec import Spec, Src0, Src1, C0, C1, maxx, lower, _has_src1
    from concourse.dve_uop import DveOpSpec

    name = "GAT_LOGIT_MASK_ANT"
    for op in dop.OPS:
        if op.name == name:
            _GAT_OP = op
            return op

    def _ref(in0, in1, s0, s1, imm2):
        v = in0.astype(np.float32) + s0
        return np.maximum(v, v * s1) + in1.astype(np.float32)

    v = Src0 + C0
    spec = Spec(body=maxx(v, v * C1) + Src1, reference=_ref)

    row = dop._CUSTOM_DVE_ROW_BASE + len(dop.OPS)
    assert row < 0x20, "custom-DVE opcode row overflow"
    uops_sha = {}
    for ver in ("v3", "v4"):
        uops = lower(spec, ver=ver)
        uops_sha[ver] = DveOpSpec(
            name=name, opcode=row, uops=uops, rd1_en=_has_src1(spec)
        ).sha(ver)
    op = dop.DveOp(name, spec, subdim=False, uops_sha=uops_sha)
    dop.OPS.append(op)
    dop._SUB_OPCODE_FOR_NAME[name] = row
    dop.CUSTOM_DVE_SPECS[name] = spec
    _GAT_OP = op
    return op


# ------------------------------------------------------------- bass program
_PROGRAM = None


def _build_program():
    """One SPMD program; per-core behavior differs only through input data."""
    global _PROGRAM
    if _PROGRAM is not None:
        return _PROGRAM

    nc = bacc.Bacc(get_trn_type() or "TRN2", target_bir_lowering=False)
    act = mybir.ActivationFunctionType

    # x pre-tiled on host: xtiles[nt, k, f, n] = x[nt*128 + n, k*128 + f]
    xt_d = nc.dram_tensor("xtiles", [NT, KC * 128, 128], DT_PE, kind="ExternalInput")
    # x-shard transposed: xsT[f, i] = x[shard_start + i, f]
    xs_d = nc.dram_tensor("xsT", [IN_F, SHARD], DT_PE, kind="ExternalInput")
    # W_ext = [W | Ws | Wt]  (Ws/Wt fold a_src/a_dst per head)
    w_d = nc.dram_tensor("wext", [IN_F, WCOLS], DT_PE, kind="ExternalInput")
    # additive log-mask, transposed + sharded: (adj.T[:, shard] - 1) * 9e15
    m_d = nc.dram_tensor("maskT", [N, SHARD], BF16, kind="ExternalInput")
    out_d = nc.dram_tensor("out", [SHARD, HEADS * OUT_F], F32, kind="ExternalOutput")

    if USE_CUSTOM_DVE:
        gat_op = _register_custom_op()

    with tile.TileContext(nc) as tc:
        with (
            tc.tile_pool(name="const", bufs=1) as cp,
            tc.tile_pool(name="xstream", bufs=3) as xp,
            tc.tile_pool(name="hpool", bufs=1) as hp,
            tc.tile_pool(name="mpool", bufs=1) as mp,
            tc.tile_pool(name="work", bufs=3) as wp,
            tc.tile_pool(name="endp", bufs=2) as ep,
            tc.tile_pool(name="ps", bufs=2, space="PSUM") as psp,
            tc.tile_pool(name="psacc", bufs=1, space="PSUM") as psa,
        ):
            # ---------------- phase A: constants in
            # phase-C critical path first: x-shard + the tiny s/t weight block
            xst = []
            for k in range(KC):
                xs_t = cp.tile([128, SHARD], DT_PE, name=f"xst{k}", tag=f"xst{k}")
                nc.sync.dma_start(xs_t, xs_d[k * 128 : (k + 1) * 128, :])
                xst.append(xs_t)
            wst = []
            for k in range(KC):
                ws_t = cp.tile([128, 2 * HEADS], DT_PE, name=f"wst{k}", tag=f"wst{k}")
                nc.sync.dma_start(ws_t, w_d[k * 128 : (k + 1) * 128, HEADS * OUT_F :])
                wst.append(ws_t)
            wsb = []
            for k in range(KC):
                w_t = cp.tile([128, WCOLS], DT_PE, name=f"wsb{k}", tag=f"wsb{k}")
                nc.sync.dma_start(w_t, w_d[k * 128 : (k + 1) * 128, :])
                wsb.append(w_t)
            ident = cp.tile([128, 128], F32, name="ident", tag="ident")
            from concourse.masks import make_identity

            make_identity(nc, ident)
            # warm the ACT exp table while DMAs land
            exp_warm = cp.tile([1, 128], F32, name="exp_warm", tag="exp_warm")
            nc.scalar.activation(exp_warm, ident[0:1, :], act.Exp)
            # memset can't write float32r; stage ones in f32, ACT-copy over
            ones_row_f32 = cp.tile([1, 128], F32, name="ones_row_f32",
                                   tag="ones_row_f32")
            nc.gpsimd.memset(ones_row_f32, 1.0)
            ones_row = cp.tile([1, 128], DT_PE, name="ones_row", tag="ones_row")
            nc.scalar.copy(ones_row, ones_row_f32)
            ones_col_f32 = cp.tile([128, HEADS], F32, name="ones_col_f32",
                                   tag="ones_col_f32")
            nc.gpsimd.memset(ones_col_f32, 1.0)

            # mask tiles: 8 big tiles of 4 j-tiles each. Only the first two
            # DMAs are issued up front; the rest interleave with the main loop
            # so they don't delay the x tiles on the sync DMA queue.
            msb = [
                mp.tile([128, 4 * SHARD], BF16, name=f"msb{b}", tag=f"msb{b}")
                for b in range(NT // 4)
            ]

            def load_mask(b):
                nc.sync.dma_start(
                    msb[b].rearrange("p (q i) -> p q i", q=4),
                    m_d[b * 512 : (b + 1) * 512, :].rearrange(
                        "(q p) i -> p q i", p=128
                    ),
                )


            # ---------------- phase C: s broadcast tile for this core's shard
            # st_k[0, i] = s[shard_start + i, k]  (one [1, SHARD] psum per head
            # so the ACT copy reads at partition 0 — offset reads are illegal)
            st_sb = [
                cp.tile([1, SHARD], DT_PE, name=f"st_sb{k}", tag=f"st_sb{k}")
                for k in range(HEADS)
            ]
            for k in range(HEADS):
                st_ps = psp.tile([1, SHARD], F32, name="st_ps", tag="pstmp")
                for kc in range(KC):
                    nc.tensor.matmul(
                        st_ps,
                        lhsT=wst[kc][:, k : k + 1],
                        rhs=xst[kc],
                        start=(kc == 0),
                        stop=(kc == KC - 1),
                    )
                nc.scalar.copy(st_sb[k], st_ps)
            # broadcast s rows across all 128 partitions via ones-matmul
            s_b4 = cp.tile([128, HEADS * SHARD], F32, name="s_b4", tag="s_b4")
            for k in range(HEADS):
                sb_ps = psp.tile([128, SHARD], F32, name="sb_ps", tag="pstmp")
                nc.tensor.matmul(
                    sb_ps,
                    lhsT=ones_row,
                    rhs=st_sb[k],
                    start=True,
                    stop=True,
                )
                nc.scalar.copy(s_b4[:, k * SHARD : (k + 1) * SHARD], sb_ps)

            # ---------------- main loop: h-compute (B) + attention (D) per tile
            acc = [
                psa.tile([HB, SHARD], F32, name=f"acc{k}", tag=f"acc{k}")
                for k in range(HEADS)
            ]
            h_sb = []
            for nt in range(NT):
                # B: h tile = x[nt*128:(nt+1)*128, :] @ W_ext   -> [128, 264]
                x_t = xp.tile([128, KC * 128], DT_PE, name="x_t", tag="x_t")
                nc.sync.dma_start(
                    x_t.rearrange("p (k n) -> p k n", n=128),
                    xt_d[nt].rearrange("(k p) n -> p k n", p=128),
                )
                # mask DMAs ride behind the x tile that needs them; issuing
                # them here (not all up front) keeps the first x tiles early
                # in the sync-DMA queue. msb[b] must be issued before the
                # jt=4b custom ops that read it.
                if nt == 0:
                    load_mask(0)
                    load_mask(1)
                if nt % 4 == 0 and 2 + nt // 4 < NT // 4:
                    load_mask(2 + nt // 4)
                ph = psp.tile([128, WCOLS], F32, name="ph", tag="ph")
                for k in range(KC):
                    nc.tensor.matmul(
                        ph,
                        lhsT=x_t[:, k * 128 : (k + 1) * 128],
                        rhs=wsb[k],
                        start=(k == 0),
                        stop=(k == KC - 1),
                    )
                # copy into packed weights layout: [h0|1|h1|1|h2|1|h3|1]
                h_t = hp.tile([128, HEADS * HB], DT_PE,
                              name=f"h_sb{nt}", tag=f"h_sb{nt}")
                nc.gpsimd.tensor_copy(
                    h_t.rearrange("p (h c) -> p h c", c=HB)[
                        :, :, OUT_F : OUT_F + 1
                    ],
                    ones_col_f32.rearrange("p (h c) -> p h c", c=1),
                )
                nc.scalar.copy(
                    h_t.rearrange("p (h c) -> p h c", c=HB)[:, :, :OUT_F],
                    ph[:, : HEADS * OUT_F].rearrange("p (h c) -> p h c", c=OUT_F),
                )
                t_sb = hp.tile([128, HEADS], F32, name=f"t_sb{nt}", tag=f"t_sb{nt}")
                nc.scalar.copy(t_sb, ph[:, HEADS * OUT_F + HEADS :])
                h_sb.append(h_t)

                # D: attention for j-tile nt (for this core's 512 i's, 4 heads)
                jt = nt
                lg = wp.tile([128, HEADS * SHARD], F32, name="lg", tag="lg")
                if USE_CUSTOM_DVE:
                    for k in range(HEADS):
                        nc.vector._custom_dve(
                            gat_op,
                            out=lg[:, k * SHARD : (k + 1) * SHARD],
                            in0=s_b4[:, k * SHARD : (k + 1) * SHARD],
                            in1=msb[jt // 4][:, (jt % 4) * SHARD : (jt % 4 + 1) * SHARD],
                            s0=t_sb[:, k : k + 1],
                            s1=ALPHA,
                        )
                else:
                    for k in range(HEADS):
                        lg_k = lg[:, k * SHARD : (k + 1) * SHARD]
                        nc.scalar.activation(
                            lg_k,
                            s_b4[:, k * SHARD : (k + 1) * SHARD],
                            act.Lrelu,
                            bias=t_sb[:, k : k + 1],
                            scale=1.0,
                            alpha=ALPHA,
                        )
                        nc.vector.tensor_add(
                            lg_k,
                            lg_k,
                            msb[jt // 4][:, (jt % 4) * SHARD : (jt % 4 + 1) * SHARD],
                        )
                at = wp.tile([128, HEADS * SHARD], DT_PE, name="at", tag="at")
                nc.scalar.activation(at, lg, act.Exp)
                for k in range(HEADS):
                    nc.tensor.matmul(
                        acc[k],
                        lhsT=h_t[:, k * HB : (k + 1) * HB],
                        rhs=at[:, k * SHARD : (k + 1) * SHARD],
                        start=(jt == 0),
                        stop=(jt == NT - 1),
                    )

            # ---------------- endgame: transpose, normalize, store
            out_sb = [
                ep.tile([128, HEADS * OUT_F], F32, name=f"osb{c}", tag=f"osb{c}",
                        bufs=1)
                for c in range(SHARD // 128)
            ]
            for k in range(HEADS):
                num_sb = ep.tile([HB, SHARD], F32, name="num_sb", tag="num_sb")
                nc.scalar.copy(num_sb, acc[k])
                for c in range(SHARD // 128):
                    tp = psp.tile([128, HB], F32, name="tp", tag="pstmp")
                    nc.tensor.transpose(
                        tp, num_sb[:, c * 128 : (c + 1) * 128], ident[:HB, :HB]
                    )
                    rec = ep.tile([128, 1], F32, name="rec", tag="rec", bufs=4)
                    nc.vector.reciprocal(rec, tp[:, OUT_F : OUT_F + 1])
                    nc.vector.tensor_scalar_mul(
                        out_sb[c][:, k * OUT_F : (k + 1) * OUT_F],
                        tp[:, :OUT_F],
                        rec,
                    )
            for c in range(SHARD // 128):
                nc.sync.dma_start(out_d[c * 128 : (c + 1) * 128, :], out_sb[c])

    nc.finalize()
    _PROGRAM = nc
    return nc


# ------------------------------------------------------------------- driver
LAST_RESULT = None


def kernel(x, adj, W, a):
    global LAST_RESULT
    x = np.asarray(x, dtype=np.float32)
    adj = np.asarray(adj)
    W = np.asarray(W, dtype=np.float32)
    a = np.asarray(a, dtype=np.float32)

    # ---- host-side layout prep (sharding + transposes, no math on the data
    # beyond folding the tiny attention vectors into W)
    a_src = a[:OUT_F, 0]
    a_dst = a[OUT_F:, 0]
    Wh = W.reshape(IN_F, HEADS, OUT_F)
    Ws = np.einsum("fhc,c->fh", Wh, a_src)       # [IN_F, HEADS]
    Wt = np.einsum("fhc,c->fh", Wh, a_dst)
    wext = np.ascontiguousarray(
        np.concatenate([W, Ws, Wt], axis=1), dtype=np.float32
    )                                            # [512, 264]

    # xtiles[nt, k, f, n] = x[nt*128 + n, k*128 + f]
    xtiles = np.ascontiguousarray(
        x.reshape(NT, 128, KC, 128).transpose(0, 2, 3, 1)
    ).reshape(NT, KC * 128, 128)

    xT = np.ascontiguousarray(x.T)               # [512, 4096]
    logmaskT = ((adj.T != 0).astype(np.float32) - 1.0) * -NEG_BIG
    logmaskT = logmaskT.astype(ml_dtypes.bfloat16)  # {0, -9e15}, exact enough

    in_maps = []
    for c in range(NCORES):
        sl = slice(c * SHARD, (c + 1) * SHARD)
        in_maps.append(
            {
                "xtiles": xtiles,
                "xsT": np.ascontiguousarray(xT[:, sl]),
                "wext": wext,
                "maskT": np.ascontiguousarray(logmaskT[:, sl]),
            }
        )

    nc = _build_program()
    res = bass_utils.run_bass_kernel_spmd(
        nc,
        in_maps,
        core_ids=list(range(NCORES)),
        trace=os.environ.get("GAT_TRACE", "0") == "1",
    )
    LAST_RESULT = res
    out = np.concatenate([r["out"] for r in res.results], axis=0)
    return out.astype(np.float32)

